# revision 17
# baseline (speedup 1.0000x reference)
"""Trainium2 Bass kernel for ConnectomeNetwork (gnn_message_passing).

Computation (reference):
    out = x @ W_retina^T                      # [B, N], nonzero only at vis cols
    for _ in range(n_layers): out = out @ W_shared^T
    y = out @ W_rational^T                    # [B, 2], reads only rat cols

The readout is rank-2, so the last n_layers-1 shared matmuls are evaluated
right-to-left as a 2-row "v-chain" (v0 = M = W_rational[:,rat] @ W_shared[rat,:],
v_{i+1} = v_i @ W_shared), while the forward side only computes
a1 = retina(x) @ W_shared^T (contracting over the 2048 vis coords).  Then
y = a1 @ v_last^T, which is column-shard-local per core: the only collective
in the kernel is a 6 KB AllGather of the 2-row v vector between v-layers.

Per core (8-way column sharding of W_shared):
  * w2 = W_shared[:, shard] in fp8-e3m4, ~18.9 MB, streamed ONCE into a
    resident SBUF tile and reused by every v-layer (the all-positive
    uniform*mask structure of W_shared makes fp8 weight noise negligible).
  * v-layer matmuls keep the 2-row (zero-padded to 32) v^T chunks stationary
    and stream the resident weights 512-wide; the three 512-column chunks of
    the shard run concurrently in three PE column-groups (tile_position).
  * retina (bf16) and L1 (fp8) weights follow w2 (and w1 precedes wret, so
    the retina/L1 compute lands inside the collective window, keeping the
    v1-tail transposes uncontended on the PE).
  * a dummy 512 B AllGather fires at t~2us under the stream: it absorbs
    launch skew and pre-warms the collective rings, cutting the real
    AllGather from ~40us to ~13us.
"""

import contextlib
import ctypes
import os

import numpy as np
import ml_dtypes

NCORES = 8
PART = 128
SLAB_KC = 8               # 128-row k-chunks per w2/w1 DMA slab
RET_CHUNK = 256           # retina vis-columns per streamed wret chunk

bf16_np = ml_dtypes.bfloat16
f8_np = ml_dtypes.float8_e3m4

_compiled_cache = {}


# --------------------------------------------------------------------------
# optional NTFF profiling hook (active only when BASS_KERNEL_PROFILE_DIR set)
# --------------------------------------------------------------------------
def _profile_ctx():
    out_dir = os.environ.get("BASS_KERNEL_PROFILE_DIR")
    if not out_dir:
        return contextlib.nullcontext()
    try:
        lib = ctypes.CDLL("/opt/axon/libaxon_pjrt.so")
        if not hasattr(lib, "axon_start_nrt_profile"):
            return contextlib.nullcontext()
        lib.axon_start_nrt_profile.argtypes = [
            ctypes.POINTER(ctypes.c_int64),
            ctypes.c_size_t,
        ]
        lib.axon_start_nrt_profile.restype = ctypes.c_int64
        lib.axon_stop_nrt_profile.argtypes = [ctypes.c_char_p]
        lib.axon_stop_nrt_profile.restype = ctypes.c_int64
    except OSError:
        return contextlib.nullcontext()

    @contextlib.contextmanager
    def _hook():
        import jax

        jax.devices()
        ids_env = os.environ.get("BASS_KERNEL_PROFILE_CORES", "")
        if ids_env:
            ids_list = [int(t) for t in ids_env.split(",") if t != ""]
            ids = (ctypes.c_int64 * len(ids_list))(*ids_list)
            rc = lib.axon_start_nrt_profile(ids, len(ids_list))
        else:
            rc = lib.axon_start_nrt_profile(None, 0)
        if rc != 0:
            raise RuntimeError(f"axon_start_nrt_profile rc={rc}")
        try:
            yield
        finally:
            os.makedirs(out_dir, exist_ok=True)
            n = lib.axon_stop_nrt_profile(str(out_dir).encode())
            print(f"profile: {n} file(s) written to {out_dir}")

    return _hook()


def _ensure_axon_platform():
    import jax

    try:
        devs = jax.devices()
    except Exception:
        devs = []
    if len(devs) >= NCORES and all("cpu" not in str(d).lower() for d in devs[:NCORES]):
        return
    import jax.extend.backend as jeb

    jeb.clear_backends()
    jax.config.update("jax_platforms", None)
    devs = jax.devices()
    if len(devs) < NCORES:
        raise RuntimeError(f"need {NCORES} neuron cores, got {devs}")


# --------------------------------------------------------------------------
# device program
# --------------------------------------------------------------------------
def _build_program(B, R, N, NVIS, n_layers):
    import concourse.bacc as bacc
    import concourse.tile as tile
    import concourse.mybir as mybir
    from concourse import masks

    bf16 = mybir.dt.bfloat16
    f8 = mybir.dt.float8e3
    f32 = mybir.dt.float32

    MSH = N // NCORES
    KC_R = R // PART            # retina contraction chunks (8)
    KC_VIS = NVIS // PART       # L1 contraction chunks (16)
    KC_N = N // PART            # full contraction chunks (96)
    MT12 = MSH // PART          # 128-col chunks of the local shard (12)
    NMC = 3                     # col-groups per v/L1 layer
    MCW = MSH // NMC            # col-group width (512)
    NSLAB = KC_N // SLAB_KC     # w2 slabs (24)
    NSLAB1 = KC_VIS // SLAB_KC  # w1 slabs (4)
    NRCH = NVIS // RET_CHUNK    # retina wret chunks (8)
    RMT = RET_CHUNK // PART     # 128-col tiles per wret chunk (2)
    n_v = n_layers - 2          # v-chain layers (2 for n_layers=4)
    assert MSH % NMC == 0 and MCW <= 512 and N % PART == 0 and R % PART == 0

    nc = bacc.Bacc("TRN2", target_bir_lowering=False, debug=False,
                   num_devices=NCORES)

    xT_d = nc.dram_tensor("xT", [PART, KC_R, B], bf16, kind="ExternalInput")
    mT_d = nc.dram_tensor("mT", [PART, KC_N, 32], bf16, kind="ExternalInput")
    mTloc_d = nc.dram_tensor("mTloc", [PART, MT12, 2], bf16,
                             kind="ExternalInput")
    w2_d = nc.dram_tensor("w2", [NSLAB, PART, SLAB_KC * MSH], f8,
                          kind="ExternalInput")
    wret_d = nc.dram_tensor("wret", [NRCH, PART, KC_R * RET_CHUNK], bf16,
                            kind="ExternalInput")
    w1_d = nc.dram_tensor("w1", [NSLAB1, PART, SLAB_KC * MSH], f8,
                          kind="ExternalInput")
    y_d = nc.dram_tensor("y_part", [B, 2], f32, kind="ExternalOutput")

    n_ags = max(n_v - 1, 0)
    ag_outs = [
        nc.dram_tensor(f"ag_out{i}", [NCORES, PART, MT12 * 2], bf16,
                       addr_space="Shared")
        for i in range(n_ags)
    ]
    ag_warm = nc.dram_tensor("ag_warm", [NCORES, PART, 2], bf16,
                             addr_space="Shared") if n_ags else None

    with tile.TileContext(nc) as tc:
        with (
            tc.tile_pool(name="const", bufs=1) as const,
            tc.tile_pool(name="wretp", bufs=2) as wretp,
            tc.tile_pool(name="psum", bufs=1, space="PSUM") as psum,
            tc.tile_pool(name="dram", bufs=1, space="DRAM") as dram,
        ):
            # ---- constants / identities ---------------------------------
            xT = const.tile([PART, KC_R, B], bf16, name="xT")
            nc.scalar.dma_start(xT[:], xT_d[:])
            mT = const.tile([PART, KC_N, 32], bf16, name="mT")
            nc.scalar.dma_start(mT[:], mT_d[:])
            mTloc = const.tile([PART, MT12, 2], bf16, name="mTloc")
            nc.scalar.dma_start(mTloc[:], mTloc_d[:])
            id2 = const.tile([2, 2], bf16, name="id2")
            masks.make_identity(nc, id2[:])
            id32 = const.tile([32, 32], bf16, name="id32")
            masks.make_identity(nc, id32[:])

            vT_fulls = []
            for i in range(n_ags):
                t = const.tile([PART, KC_N, 32], bf16, name=f"vT_full{i}")
                nc.vector.memset(t[:], 0.0)
                vT_fulls.append(t)

            # warm-up collective: absorbs launch skew and pre-warms the
            # CC rings under the w2 stream (gpsimd is otherwise idle here)
            if ag_warm is not None:
                warm_sb = const.tile([PART, 2], bf16, name="warm_sb")
                nc.vector.memset(warm_sb[:], 0.0)
                warm_in = dram.tile([PART, 2], bf16, name="warm_in",
                                    tag="warm_in")
                nc.scalar.dma_start(warm_in[:], warm_sb[:])
                nc.gpsimd.collective_compute(
                    "AllGather",
                    mybir.AluOpType.bypass,
                    replica_groups=[list(range(NCORES))],
                    ins=[warm_in.opt()],
                    outs=[ag_warm[:]],
                )
                warm_out = const.tile([PART, 2], bf16, name="warm_out")
                nc.scalar.dma_start(warm_out[:], ag_warm[0])

            def v_extract(ps, n_rows, name):
                """psum col-group slices -> v_sb [n_rows, MSH] bf16."""
                v_sb = const.tile([n_rows, MSH], bf16, name=name)
                for mc in range(NMC):
                    nc.vector.tensor_copy(
                        v_sb[:, MCW * mc:MCW * (mc + 1)],
                        ps[32 * mc:32 * mc + n_rows, :])
                return v_sb

            def v_transpose(v_sb, ident, n_rows, dst):
                """v_sb [n_rows, MSH] -> dst [PART, MT12, n_rows] via PE."""
                for t in range(MT12):
                    ps_t = psum.tile([PART, 32], bf16, name="ps_t",
                                     tag="ps_tr", bufs=2)
                    nc.tensor.transpose(
                        ps_t[:, 0:n_rows],
                        v_sb[:, t * PART:(t + 1) * PART], ident)
                    nc.vector.tensor_copy(dst[:, t, :], ps_t[:, 0:n_rows])

            # ---- w2 resident stream + v-layer-0 (trails the stream) -----
            w2_sb = const.tile([PART, KC_N, MSH], f8, name="w2_sb")
            ps_v = psum.tile([PART, MCW], f32, name="ps_v", tag="ps_v")
            for sl in range(NSLAB):
                nc.sync.dma_start(
                    w2_sb.rearrange("p k m -> p (k m)")
                    [:, sl * SLAB_KC * MSH:(sl + 1) * SLAB_KC * MSH],
                    w2_d[sl])
                if n_v >= 1:
                    for kc in range(SLAB_KC):
                        kg = sl * SLAB_KC + kc
                        for mc in range(NMC):
                            nc.tensor.matmul(
                                ps_v[32 * mc:32 * mc + 32, :],
                                mT[:, kg, :],
                                w2_sb[:, kg, MCW * mc:MCW * (mc + 1)],
                                start=(kg == 0), stop=(kg == KC_N - 1),
                                tile_position=(0, 32 * mc))

            # v-layer 0 tail: extract, transpose, AllGather (if more layers)
            if n_v >= 1:
                v_sb = v_extract(ps_v, 2, "v0_sb")
                if n_ags >= 1:
                    vT_loc = const.tile([PART, MT12, 2], bf16, name="v0T_loc")
                    v_transpose(v_sb, id2, 2, vT_loc)
                    cc_in = dram.tile([PART, MT12 * 2], bf16, name="cc_in0",
                                      tag="cc_in0")
                    nc.scalar.dma_start(
                        cc_in[:], vT_loc.rearrange("p t o -> p (t o)"))
                    nc.gpsimd.collective_compute(
                        "AllGather",
                        mybir.AluOpType.bypass,
                        replica_groups=[list(range(NCORES))],
                        ins=[cc_in.opt()],
                        outs=[ag_outs[0][:]],
                    )
                    compact = const.tile([PART, NCORES, MT12 * 2], bf16,
                                         name="compact0")
                    nc.scalar.dma_start(
                        compact[:], ag_outs[0].rearrange("r p x -> p r x"))
                    nc.vector.tensor_copy(
                        vT_fulls[0][:, :, 0:2],
                        compact.rearrange("p r (t o) -> p (r t) o", o=2))

            # ---- w1 stream (before wret: retina data arrives last) -------------------
            w1_sb = const.tile([PART, KC_VIS, MSH], f8, name="w1_sb")
            for sl in range(NSLAB1):
                nc.sync.dma_start(
                    w1_sb.rearrange("p k m -> p (k m)")
                    [:, sl * SLAB_KC * MSH:(sl + 1) * SLAB_KC * MSH],
                    w1_d[sl])

            # ---- retina: aTvis[p, kg, b] (stationary-wret form) ----------
            aTvis = const.tile([PART, KC_VIS, B], bf16, name="aTvis")
            for ch in range(NRCH):
                wret_sb = wretp.tile([PART, KC_R, RET_CHUNK], bf16,
                                     name="wret_sb", tag="wret")
                nc.sync.dma_start(
                    wret_sb.rearrange("p k m -> p (k m)"), wret_d[ch])
                for mt in range(RMT):
                    ps_r = psum.tile([PART, B], f32, name="ps_r",
                                     tag="ps_ret", bufs=2)
                    for kc in range(KC_R):
                        nc.tensor.matmul(
                            ps_r[:],
                            wret_sb[:, kc, mt * PART:(mt + 1) * PART],
                            xT[:, kc, :],
                            start=(kc == 0), stop=(kc == KC_R - 1))
                    nc.scalar.activation(aTvis[:, ch * RMT + mt, :], ps_r[:],
                                         mybir.ActivationFunctionType.Copy)

            # ---- remaining v-layers (resident w2 reuse) ------------------
            vT_last_loc = None
            for vi in range(1, n_v):
                ps_vi = psum.tile([PART, MCW], f32, name="ps_vi",
                                  tag="ps_v2")
                for kg in range(KC_N):
                    for mc in range(NMC):
                        nc.tensor.matmul(
                            ps_vi[32 * mc:32 * mc + 32, :],
                            vT_fulls[vi - 1][:, kg, :],
                            w2_sb[:, kg, MCW * mc:MCW * (mc + 1)],
                            start=(kg == 0), stop=(kg == KC_N - 1),
                            tile_position=(0, 32 * mc))
                v_sb = v_extract(ps_vi, 2, f"v{vi}_sb")
                if vi < n_v - 1:
                    # mid v-layer: transpose + AllGather to vT_fulls[vi]
                    vT_loc = const.tile([PART, MT12, 2], bf16,
                                        name=f"v{vi}T_loc")
                    v_transpose(v_sb, id2, 2, vT_loc)
                    cc_in = dram.tile([PART, MT12 * 2], bf16,
                                      name=f"cc_in{vi}", tag=f"cc_in{vi}")
                    nc.scalar.dma_start(
                        cc_in[:], vT_loc.rearrange("p t o -> p (t o)"))
                    nc.gpsimd.collective_compute(
                        "AllGather",
                        mybir.AluOpType.bypass,
                        replica_groups=[list(range(NCORES))],
                        ins=[cc_in.opt()],
                        outs=[ag_outs[vi][:]],
                    )
                    compact = const.tile([PART, NCORES, MT12 * 2], bf16,
                                         name=f"compact{vi}")
                    nc.scalar.dma_start(
                        compact[:], ag_outs[vi].rearrange("r p x -> p r x"))
                    nc.vector.tensor_copy(
                        vT_fulls[vi][:, :, 0:2],
                        compact.rearrange("p r (t o) -> p (r t) o", o=2))
                else:
                    # last v-layer stays local: transpose for the fold
                    vT_last_loc = const.tile([PART, MT12, 2], bf16,
                                             name="vT_last")
                    v_transpose(v_sb, id2, 2, vT_last_loc)
            if n_v == 1:
                vT_last_loc = const.tile([PART, MT12, 2], bf16,
                                         name="vT_last")
                v_transpose(v_sb, id2, 2, vT_last_loc)

            # ---- L1: a1[b, shard] via mc col-groups ----------------------
            ps_a = psum.tile([PART, MCW], f32, name="ps_a", tag="ps_a")
            for kg in range(KC_VIS):
                for mc in range(NMC):
                    nc.tensor.matmul(
                        ps_a[32 * mc:32 * mc + 32, :],
                        aTvis[:, kg, :],
                        w1_sb[:, kg, MCW * mc:MCW * (mc + 1)],
                        start=(kg == 0), stop=(kg == KC_VIS - 1),
                        tile_position=(0, 32 * mc))
            a1_sb = v_extract(ps_a, B, "a1_sb")
            a1T = const.tile([PART, MT12, B], bf16, name="a1T")
            v_transpose(a1_sb, id32, B, a1T)

            # ---- fold: y_part = a1_shard @ v_last_shard^T ----------------
            rhs_fold = vT_last_loc if n_v >= 1 else mTloc
            ps_y = psum.tile([B, 2], f32, name="ps_y", tag="ps_y")
            for t in range(MT12):
                nc.tensor.matmul(
                    ps_y[:], a1T[:, t, :], rhs_fold[:, t, :],
                    start=(t == 0), stop=(t == MT12 - 1))
            y_sb = const.tile([B, 2], f32, name="y_sb")
            nc.vector.tensor_copy(y_sb[:], ps_y[:])
            nc.scalar.dma_start(y_d[:], y_sb[:])

    nc.compile()
    return nc


def _slab_swizzle(w, slab_kc, dtype):
    """[rows, m] -> [n_slabs, 128, slab_kc * m]:
    out[s, p, j*m:(j+1)*m] = w[(s*slab_kc + j)*128 + p, :]."""
    rows, m = w.shape
    n_slabs = (rows + slab_kc * PART - 1) // (slab_kc * PART)
    pad_rows = n_slabs * slab_kc * PART - rows
    if pad_rows:
        w = np.concatenate([w, np.zeros((pad_rows, m), w.dtype)], axis=0)
    out = (np.ascontiguousarray(
        w.reshape(n_slabs, slab_kc, PART, m).transpose(0, 2, 1, 3))
        .reshape(n_slabs, PART, slab_kc * m))
    return out.astype(dtype)


# --------------------------------------------------------------------------
# host entry point
# --------------------------------------------------------------------------
def kernel(x, W_retina, W_shared, W_rational, n_layers):
    x = np.asarray(x, np.float32)
    W_retina = np.asarray(W_retina, np.float32)
    W_shared = np.asarray(W_shared, np.float32)
    W_rational = np.asarray(W_rational, np.float32)
    L = int(n_layers)

    B, R = x.shape
    N = W_shared.shape[0]
    O = W_rational.shape[0]

    vis = np.flatnonzero(np.any(W_retina != 0, axis=1))
    rat = np.flatnonzero(np.any(W_rational != 0, axis=0))

    if L < 2 or len(vis) == 0 or len(rat) == 0 or N % (NCORES * 512) != 0 \
            or R % PART != 0 or O != 2 or B != 32:
        out = x @ W_retina.T
        for _ in range(L):
            out = out @ W_shared.T
        return (out @ W_rational.T).astype(np.float32)

    # pad vis with zero-rows of W_retina (exact: they contribute 0)
    pad = (-len(vis)) % (SLAB_KC * PART)
    if pad:
        comp = np.setdiff1d(np.arange(N), vis, assume_unique=False)
        if len(comp) < pad:
            out = x @ W_retina.T
            for _ in range(L):
                out = out @ W_shared.T
            return (out @ W_rational.T).astype(np.float32)
        vis = np.concatenate([vis, comp[:pad]])
    NVIS = len(vis)
    MSH = N // NCORES
    KC_N = N // PART

    # ---- host-side prep --------------------------------------------------
    # M = W_rational[:, rat] @ W_shared[rat, :]  (exact fold, 64 rows touched)
    M = (W_rational[:, rat].astype(np.float64)
         @ W_shared[rat, :].astype(np.float64)).astype(np.float32)

    xT = np.ascontiguousarray(x.T).reshape(R // PART, PART, B) \
        .transpose(1, 0, 2)                                     # [128, KC_R, B]
    xT = np.ascontiguousarray(xT).astype(bf16_np)
    # mT[p, kg, o] = M[o, kg*128+p], padded to 32 cols
    mT = np.zeros((PART, KC_N, 32), np.float32)
    mT[:, :, 0:2] = M.T.reshape(KC_N, PART, 2).transpose(1, 0, 2)
    mT = mT.astype(bf16_np)

    WretT = np.ascontiguousarray(W_retina[vis].T)               # [R, NVIS]
    # wret chunks: [NRCH, 128, KC_R*RET_CHUNK]:
    # wret[ch][p, kc*RET_CHUNK+m] = WretT[kc*128+p, ch*RET_CHUNK+m]
    NRCH = NVIS // RET_CHUNK
    wret = (WretT.reshape(R // PART, PART, NRCH, RET_CHUNK)
            .transpose(2, 1, 0, 3)
            .reshape(NRCH, PART, (R // PART) * RET_CHUNK))
    wret = np.ascontiguousarray(wret).astype(bf16_np)

    Ws8 = W_shared.astype(f8_np)                                # one big cast

    w2_c, w1_c, mloc_c = [], [], []
    for c in range(NCORES):
        sl = slice(c * MSH, (c + 1) * MSH)
        w2_c.append(_slab_swizzle(np.ascontiguousarray(Ws8[:, sl]),
                                  SLAB_KC, f8_np))
        w1km = np.ascontiguousarray(Ws8[sl, :][:, vis].T)       # [NVIS, MSH]
        w1_c.append(_slab_swizzle(w1km, SLAB_KC, f8_np))
        mloc = M[:, sl].T.reshape(MSH // PART, PART, 2).transpose(1, 0, 2)
        mloc_c.append(np.ascontiguousarray(mloc).astype(bf16_np))

    _ensure_axon_platform()
    from concourse.bass_utils import run_bass_kernel_spmd

    key = (B, R, N, NVIS, L)
    if key not in _compiled_cache:
        _compiled_cache[key] = _build_program(B, R, N, NVIS, L)
    nc = _compiled_cache[key]

    in_maps = []
    for c in range(NCORES):
        in_maps.append({
            "xT": xT, "mT": mT, "mTloc": mloc_c[c],
            "w2": w2_c[c], "wret": wret, "w1": w1_c[c],
        })

    with _profile_ctx():
        res = run_bass_kernel_spmd(nc, in_maps, core_ids=list(range(NCORES)))

    y = np.zeros((B, O), np.float64)
    for c in range(NCORES):
        y += res.results[c]["y_part"].astype(np.float64)
    return y.astype(np.float32)


# revision 18
# speedup vs baseline: 1.0770x; 1.0770x over previous
"""Trainium2 Bass kernel for ConnectomeNetwork (gnn_message_passing).

Computation (reference):
    out = x @ W_retina^T                      # [B, N], nonzero only at vis cols
    for _ in range(n_layers): out = out @ W_shared^T
    y = out @ W_rational^T                    # [B, 2], reads only rat cols

The readout is rank-2, so the last n_layers-1 shared matmuls are evaluated
right-to-left as a 2-row "v-chain" (v0 = M = W_rational[:,rat] @ W_shared[rat,:],
v_{i+1} = v_i @ W_shared), while the forward side only computes
a1 = retina(x) @ W_shared^T (contracting over the 2048 vis coords).  Then
y = a1 @ v_last^T, which is column-shard-local per core: the only collective
in the kernel is a 6 KB AllGather of the 2-row v vector between v-layers.

Per core (8-way column sharding of W_shared):
  * w2 = W_shared[:, shard] in fp8-e3m4, ~18.9 MB, streamed ONCE into a
    resident SBUF tile and reused by every v-layer (the all-positive
    uniform*mask structure of W_shared makes fp8 weight noise negligible).
  * v-layer matmuls keep the 2-row (zero-padded to 32) v^T chunks stationary
    and stream the resident weights 512-wide; the three 512-column chunks of
    the shard run concurrently in three PE column-groups (tile_position).
  * retina (bf16) and L1 (fp8) weights follow w2 (and w1 precedes wret, so
    the retina/L1 compute lands inside the collective window, keeping the
    v1-tail transposes uncontended on the PE).
  * a dummy 512 B AllGather fires at t~2us under the stream: it absorbs
    launch skew and pre-warms the collective rings, cutting the real
    AllGather from ~40us to ~13us.
"""

import contextlib
import ctypes
import os

import numpy as np
import ml_dtypes

NCORES = 8
PART = 128
SLAB_KC = 4               # 128-row k-chunks per w2/w1 DMA slab
RET_CHUNK = 256           # retina vis-columns per streamed wret chunk

bf16_np = ml_dtypes.bfloat16
f8_np = ml_dtypes.float8_e3m4

_compiled_cache = {}


# --------------------------------------------------------------------------
# optional NTFF profiling hook (active only when BASS_KERNEL_PROFILE_DIR set)
# --------------------------------------------------------------------------
def _profile_ctx():
    out_dir = os.environ.get("BASS_KERNEL_PROFILE_DIR")
    if not out_dir:
        return contextlib.nullcontext()
    try:
        lib = ctypes.CDLL("/opt/axon/libaxon_pjrt.so")
        if not hasattr(lib, "axon_start_nrt_profile"):
            return contextlib.nullcontext()
        lib.axon_start_nrt_profile.argtypes = [
            ctypes.POINTER(ctypes.c_int64),
            ctypes.c_size_t,
        ]
        lib.axon_start_nrt_profile.restype = ctypes.c_int64
        lib.axon_stop_nrt_profile.argtypes = [ctypes.c_char_p]
        lib.axon_stop_nrt_profile.restype = ctypes.c_int64
    except OSError:
        return contextlib.nullcontext()

    @contextlib.contextmanager
    def _hook():
        import jax

        jax.devices()
        ids_env = os.environ.get("BASS_KERNEL_PROFILE_CORES", "")
        if ids_env:
            ids_list = [int(t) for t in ids_env.split(",") if t != ""]
            ids = (ctypes.c_int64 * len(ids_list))(*ids_list)
            rc = lib.axon_start_nrt_profile(ids, len(ids_list))
        else:
            rc = lib.axon_start_nrt_profile(None, 0)
        if rc != 0:
            raise RuntimeError(f"axon_start_nrt_profile rc={rc}")
        try:
            yield
        finally:
            os.makedirs(out_dir, exist_ok=True)
            n = lib.axon_stop_nrt_profile(str(out_dir).encode())
            print(f"profile: {n} file(s) written to {out_dir}")

    return _hook()


def _ensure_axon_platform():
    import jax

    try:
        devs = jax.devices()
    except Exception:
        devs = []
    if len(devs) >= NCORES and all("cpu" not in str(d).lower() for d in devs[:NCORES]):
        return
    import jax.extend.backend as jeb

    jeb.clear_backends()
    jax.config.update("jax_platforms", None)
    devs = jax.devices()
    if len(devs) < NCORES:
        raise RuntimeError(f"need {NCORES} neuron cores, got {devs}")


# --------------------------------------------------------------------------
# device program
# --------------------------------------------------------------------------
def _build_program(B, R, N, NVIS, n_layers):
    import concourse.bacc as bacc
    import concourse.tile as tile
    import concourse.mybir as mybir
    from concourse import masks

    bf16 = mybir.dt.bfloat16
    f8 = mybir.dt.float8e3
    f32 = mybir.dt.float32

    MSH = N // NCORES
    KC_R = R // PART            # retina contraction chunks (8)
    KC_VIS = NVIS // PART       # L1 contraction chunks (16)
    KC_N = N // PART            # full contraction chunks (96)
    MT12 = MSH // PART          # 128-col chunks of the local shard (12)
    NMC = 3                     # col-groups per v/L1 layer
    MCW = MSH // NMC            # col-group width (512)
    NSLAB = KC_N // SLAB_KC     # w2 slabs (24)
    NSLAB1 = KC_VIS // SLAB_KC  # w1 slabs (4)
    NRCH = NVIS // RET_CHUNK    # retina wret chunks (8)
    RMT = RET_CHUNK // PART     # 128-col tiles per wret chunk (2)
    n_v = n_layers - 2          # v-chain layers (2 for n_layers=4)
    assert MSH % NMC == 0 and MCW <= 512 and N % PART == 0 and R % PART == 0

    nc = bacc.Bacc("TRN2", target_bir_lowering=False, debug=False,
                   num_devices=NCORES)

    xT_d = nc.dram_tensor("xT", [PART, KC_R, B], bf16, kind="ExternalInput")
    mT_d = nc.dram_tensor("mT", [PART, KC_N, 32], bf16, kind="ExternalInput")
    mTloc_d = nc.dram_tensor("mTloc", [PART, MT12, 2], bf16,
                             kind="ExternalInput")
    w2_d = nc.dram_tensor("w2", [NSLAB, PART, SLAB_KC * MSH], f8,
                          kind="ExternalInput")
    wret_d = nc.dram_tensor("wret", [NRCH, PART, KC_R * RET_CHUNK], bf16,
                            kind="ExternalInput")
    w1_d = nc.dram_tensor("w1", [NSLAB1, PART, SLAB_KC * MSH], f8,
                          kind="ExternalInput")
    y_d = nc.dram_tensor("y_part", [B, 2], f32, kind="ExternalOutput")

    n_ags = max(n_v - 1, 0)
    ag_outs = [
        nc.dram_tensor(f"ag_out{i}", [NCORES, PART, MT12 * 2], bf16,
                       addr_space="Shared")
        for i in range(n_ags)
    ]
    ag_warm = nc.dram_tensor("ag_warm", [NCORES, PART, 2], bf16,
                             addr_space="Shared") if n_ags else None

    with tile.TileContext(nc) as tc:
        with (
            tc.tile_pool(name="const", bufs=1) as const,
            tc.tile_pool(name="wretp", bufs=2) as wretp,
            tc.tile_pool(name="psum", bufs=1, space="PSUM") as psum,
            tc.tile_pool(name="dram", bufs=1, space="DRAM") as dram,
        ):
            # ---- constants / identities ---------------------------------
            xT = const.tile([PART, KC_R, B], bf16, name="xT")
            nc.scalar.dma_start(xT[:], xT_d[:])
            mT = const.tile([PART, KC_N, 32], bf16, name="mT")
            nc.scalar.dma_start(mT[:], mT_d[:])
            mTloc = const.tile([PART, MT12, 2], bf16, name="mTloc")
            nc.scalar.dma_start(mTloc[:], mTloc_d[:])
            id2 = const.tile([2, 2], bf16, name="id2")
            masks.make_identity(nc, id2[:])
            id32 = const.tile([32, 32], bf16, name="id32")
            masks.make_identity(nc, id32[:])

            vT_fulls = []
            for i in range(n_ags):
                t = const.tile([PART, KC_N, 32], bf16, name=f"vT_full{i}")
                nc.vector.memset(t[:], 0.0)
                vT_fulls.append(t)

            # warm-up collective: absorbs launch skew and pre-warms the
            # CC rings under the w2 stream (gpsimd is otherwise idle here)
            if ag_warm is not None:
                warm_sb = const.tile([PART, 2], bf16, name="warm_sb")
                nc.vector.memset(warm_sb[:], 0.0)
                warm_in = dram.tile([PART, 2], bf16, name="warm_in",
                                    tag="warm_in")
                nc.scalar.dma_start(warm_in[:], warm_sb[:])
                nc.gpsimd.collective_compute(
                    "AllGather",
                    mybir.AluOpType.bypass,
                    replica_groups=[list(range(NCORES))],
                    ins=[warm_in.opt()],
                    outs=[ag_warm[:]],
                )
                warm_out = const.tile([PART, 2], bf16, name="warm_out")
                nc.scalar.dma_start(warm_out[:], ag_warm[0])

            def v_extract(ps, n_rows, name):
                """psum col-group slices -> v_sb [n_rows, MSH] bf16."""
                v_sb = const.tile([n_rows, MSH], bf16, name=name)
                for mc in range(NMC):
                    nc.vector.tensor_copy(
                        v_sb[:, MCW * mc:MCW * (mc + 1)],
                        ps[32 * mc:32 * mc + n_rows, :])
                return v_sb

            def v_transpose(v_sb, ident, n_rows, dst):
                """v_sb [n_rows, MSH] -> dst [PART, MT12, n_rows] via PE."""
                for t in range(MT12):
                    ps_t = psum.tile([PART, 32], bf16, name="ps_t",
                                     tag="ps_tr", bufs=2)
                    nc.tensor.transpose(
                        ps_t[:, 0:n_rows],
                        v_sb[:, t * PART:(t + 1) * PART], ident)
                    nc.vector.tensor_copy(dst[:, t, :], ps_t[:, 0:n_rows])

            # ---- w2 resident stream + v-layer-0 (trails the stream) -----
            w2_sb = const.tile([PART, KC_N, MSH], f8, name="w2_sb")
            ps_v = psum.tile([PART, MCW], f32, name="ps_v", tag="ps_v")
            for sl in range(NSLAB):
                nc.sync.dma_start(
                    w2_sb.rearrange("p k m -> p (k m)")
                    [:, sl * SLAB_KC * MSH:(sl + 1) * SLAB_KC * MSH],
                    w2_d[sl])
                if n_v >= 1:
                    for kc in range(SLAB_KC):
                        kg = sl * SLAB_KC + kc
                        for mc in range(NMC):
                            nc.tensor.matmul(
                                ps_v[32 * mc:32 * mc + 32, :],
                                mT[:, kg, :],
                                w2_sb[:, kg, MCW * mc:MCW * (mc + 1)],
                                start=(kg == 0), stop=(kg == KC_N - 1),
                                tile_position=(0, 32 * mc))

            # v-layer 0 tail: extract, transpose, AllGather (if more layers)
            if n_v >= 1:
                v_sb = v_extract(ps_v, 2, "v0_sb")
                if n_ags >= 1:
                    vT_loc = const.tile([PART, MT12, 2], bf16, name="v0T_loc")
                    v_transpose(v_sb, id2, 2, vT_loc)
                    cc_in = dram.tile([PART, MT12 * 2], bf16, name="cc_in0",
                                      tag="cc_in0")
                    nc.scalar.dma_start(
                        cc_in[:], vT_loc.rearrange("p t o -> p (t o)"))
                    nc.gpsimd.collective_compute(
                        "AllGather",
                        mybir.AluOpType.bypass,
                        replica_groups=[list(range(NCORES))],
                        ins=[cc_in.opt()],
                        outs=[ag_outs[0][:]],
                    )
                    compact = const.tile([PART, NCORES, MT12 * 2], bf16,
                                         name="compact0")
                    nc.scalar.dma_start(
                        compact[:], ag_outs[0].rearrange("r p x -> p r x"))
                    nc.vector.tensor_copy(
                        vT_fulls[0][:, :, 0:2],
                        compact.rearrange("p r (t o) -> p (r t) o", o=2))

            # ---- w1 stream (before wret: retina data arrives last) -------------------
            w1_sb = const.tile([PART, KC_VIS, MSH], f8, name="w1_sb")
            for sl in range(NSLAB1):
                nc.sync.dma_start(
                    w1_sb.rearrange("p k m -> p (k m)")
                    [:, sl * SLAB_KC * MSH:(sl + 1) * SLAB_KC * MSH],
                    w1_d[sl])

            # ---- retina: aTvis[p, kg, b] (stationary-wret form) ----------
            aTvis = const.tile([PART, KC_VIS, B], bf16, name="aTvis")
            for ch in range(NRCH):
                wret_sb = wretp.tile([PART, KC_R, RET_CHUNK], bf16,
                                     name="wret_sb", tag="wret")
                nc.sync.dma_start(
                    wret_sb.rearrange("p k m -> p (k m)"), wret_d[ch])
                for mt in range(RMT):
                    ps_r = psum.tile([PART, B], f32, name="ps_r",
                                     tag="ps_ret", bufs=2)
                    for kc in range(KC_R):
                        nc.tensor.matmul(
                            ps_r[:],
                            wret_sb[:, kc, mt * PART:(mt + 1) * PART],
                            xT[:, kc, :],
                            start=(kc == 0), stop=(kc == KC_R - 1))
                    nc.scalar.activation(aTvis[:, ch * RMT + mt, :], ps_r[:],
                                         mybir.ActivationFunctionType.Copy)

            # ---- remaining v-layers (resident w2 reuse) ------------------
            vT_last_loc = None
            for vi in range(1, n_v):
                ps_vi = psum.tile([PART, MCW], f32, name="ps_vi",
                                  tag="ps_v2")
                for kg in range(KC_N):
                    for mc in range(NMC):
                        nc.tensor.matmul(
                            ps_vi[32 * mc:32 * mc + 32, :],
                            vT_fulls[vi - 1][:, kg, :],
                            w2_sb[:, kg, MCW * mc:MCW * (mc + 1)],
                            start=(kg == 0), stop=(kg == KC_N - 1),
                            tile_position=(0, 32 * mc))
                v_sb = v_extract(ps_vi, 2, f"v{vi}_sb")
                if vi < n_v - 1:
                    # mid v-layer: transpose + AllGather to vT_fulls[vi]
                    vT_loc = const.tile([PART, MT12, 2], bf16,
                                        name=f"v{vi}T_loc")
                    v_transpose(v_sb, id2, 2, vT_loc)
                    cc_in = dram.tile([PART, MT12 * 2], bf16,
                                      name=f"cc_in{vi}", tag=f"cc_in{vi}")
                    nc.scalar.dma_start(
                        cc_in[:], vT_loc.rearrange("p t o -> p (t o)"))
                    nc.gpsimd.collective_compute(
                        "AllGather",
                        mybir.AluOpType.bypass,
                        replica_groups=[list(range(NCORES))],
                        ins=[cc_in.opt()],
                        outs=[ag_outs[vi][:]],
                    )
                    compact = const.tile([PART, NCORES, MT12 * 2], bf16,
                                         name=f"compact{vi}")
                    nc.scalar.dma_start(
                        compact[:], ag_outs[vi].rearrange("r p x -> p r x"))
                    nc.vector.tensor_copy(
                        vT_fulls[vi][:, :, 0:2],
                        compact.rearrange("p r (t o) -> p (r t) o", o=2))
                else:
                    # last v-layer stays local: transpose for the fold
                    vT_last_loc = const.tile([PART, MT12, 2], bf16,
                                             name="vT_last")
                    v_transpose(v_sb, id2, 2, vT_last_loc)
            if n_v == 1:
                vT_last_loc = const.tile([PART, MT12, 2], bf16,
                                         name="vT_last")
                v_transpose(v_sb, id2, 2, vT_last_loc)

            # ---- L1: a1[b, shard] via mc col-groups ----------------------
            ps_a = psum.tile([PART, MCW], f32, name="ps_a", tag="ps_a")
            for kg in range(KC_VIS):
                for mc in range(NMC):
                    nc.tensor.matmul(
                        ps_a[32 * mc:32 * mc + 32, :],
                        aTvis[:, kg, :],
                        w1_sb[:, kg, MCW * mc:MCW * (mc + 1)],
                        start=(kg == 0), stop=(kg == KC_VIS - 1),
                        tile_position=(0, 32 * mc))
            a1_sb = v_extract(ps_a, B, "a1_sb")
            a1T = const.tile([PART, MT12, B], bf16, name="a1T")
            v_transpose(a1_sb, id32, B, a1T)

            # ---- fold: y_part = a1_shard @ v_last_shard^T ----------------
            rhs_fold = vT_last_loc if n_v >= 1 else mTloc
            ps_y = psum.tile([B, 2], f32, name="ps_y", tag="ps_y")
            for t in range(MT12):
                nc.tensor.matmul(
                    ps_y[:], a1T[:, t, :], rhs_fold[:, t, :],
                    start=(t == 0), stop=(t == MT12 - 1))
            y_sb = const.tile([B, 2], f32, name="y_sb")
            nc.vector.tensor_copy(y_sb[:], ps_y[:])
            nc.scalar.dma_start(y_d[:], y_sb[:])

    nc.compile()
    return nc


def _slab_swizzle(w, slab_kc, dtype):
    """[rows, m] -> [n_slabs, 128, slab_kc * m]:
    out[s, p, j*m:(j+1)*m] = w[(s*slab_kc + j)*128 + p, :]."""
    rows, m = w.shape
    n_slabs = (rows + slab_kc * PART - 1) // (slab_kc * PART)
    pad_rows = n_slabs * slab_kc * PART - rows
    if pad_rows:
        w = np.concatenate([w, np.zeros((pad_rows, m), w.dtype)], axis=0)
    out = (np.ascontiguousarray(
        w.reshape(n_slabs, slab_kc, PART, m).transpose(0, 2, 1, 3))
        .reshape(n_slabs, PART, slab_kc * m))
    return out.astype(dtype)


# --------------------------------------------------------------------------
# host entry point
# --------------------------------------------------------------------------
def kernel(x, W_retina, W_shared, W_rational, n_layers):
    x = np.asarray(x, np.float32)
    W_retina = np.asarray(W_retina, np.float32)
    W_shared = np.asarray(W_shared, np.float32)
    W_rational = np.asarray(W_rational, np.float32)
    L = int(n_layers)

    B, R = x.shape
    N = W_shared.shape[0]
    O = W_rational.shape[0]

    vis = np.flatnonzero(np.any(W_retina != 0, axis=1))
    rat = np.flatnonzero(np.any(W_rational != 0, axis=0))

    if L < 2 or len(vis) == 0 or len(rat) == 0 or N % (NCORES * 512) != 0 \
            or R % PART != 0 or O != 2 or B != 32:
        out = x @ W_retina.T
        for _ in range(L):
            out = out @ W_shared.T
        return (out @ W_rational.T).astype(np.float32)

    # pad vis with zero-rows of W_retina (exact: they contribute 0)
    pad = (-len(vis)) % (SLAB_KC * PART)
    if pad:
        comp = np.setdiff1d(np.arange(N), vis, assume_unique=False)
        if len(comp) < pad:
            out = x @ W_retina.T
            for _ in range(L):
                out = out @ W_shared.T
            return (out @ W_rational.T).astype(np.float32)
        vis = np.concatenate([vis, comp[:pad]])
    NVIS = len(vis)
    MSH = N // NCORES
    KC_N = N // PART

    # ---- host-side prep --------------------------------------------------
    # M = W_rational[:, rat] @ W_shared[rat, :]  (exact fold, 64 rows touched)
    M = (W_rational[:, rat].astype(np.float64)
         @ W_shared[rat, :].astype(np.float64)).astype(np.float32)

    xT = np.ascontiguousarray(x.T).reshape(R // PART, PART, B) \
        .transpose(1, 0, 2)                                     # [128, KC_R, B]
    xT = np.ascontiguousarray(xT).astype(bf16_np)
    # mT[p, kg, o] = M[o, kg*128+p], padded to 32 cols
    mT = np.zeros((PART, KC_N, 32), np.float32)
    mT[:, :, 0:2] = M.T.reshape(KC_N, PART, 2).transpose(1, 0, 2)
    mT = mT.astype(bf16_np)

    WretT = np.ascontiguousarray(W_retina[vis].T)               # [R, NVIS]
    # wret chunks: [NRCH, 128, KC_R*RET_CHUNK]:
    # wret[ch][p, kc*RET_CHUNK+m] = WretT[kc*128+p, ch*RET_CHUNK+m]
    NRCH = NVIS // RET_CHUNK
    wret = (WretT.reshape(R // PART, PART, NRCH, RET_CHUNK)
            .transpose(2, 1, 0, 3)
            .reshape(NRCH, PART, (R // PART) * RET_CHUNK))
    wret = np.ascontiguousarray(wret).astype(bf16_np)

    Ws8 = W_shared.astype(f8_np)                                # one big cast

    w2_c, w1_c, mloc_c = [], [], []
    for c in range(NCORES):
        sl = slice(c * MSH, (c + 1) * MSH)
        w2_c.append(_slab_swizzle(np.ascontiguousarray(Ws8[:, sl]),
                                  SLAB_KC, f8_np))
        w1km = np.ascontiguousarray(Ws8[sl, :][:, vis].T)       # [NVIS, MSH]
        w1_c.append(_slab_swizzle(w1km, SLAB_KC, f8_np))
        mloc = M[:, sl].T.reshape(MSH // PART, PART, 2).transpose(1, 0, 2)
        mloc_c.append(np.ascontiguousarray(mloc).astype(bf16_np))

    _ensure_axon_platform()
    from concourse.bass_utils import run_bass_kernel_spmd

    key = (B, R, N, NVIS, L)
    if key not in _compiled_cache:
        _compiled_cache[key] = _build_program(B, R, N, NVIS, L)
    nc = _compiled_cache[key]

    in_maps = []
    for c in range(NCORES):
        in_maps.append({
            "xT": xT, "mT": mT, "mTloc": mloc_c[c],
            "w2": w2_c[c], "wret": wret, "w1": w1_c[c],
        })

    with _profile_ctx():
        res = run_bass_kernel_spmd(nc, in_maps, core_ids=list(range(NCORES)))

    y = np.zeros((B, O), np.float64)
    for c in range(NCORES):
        y += res.results[c]["y_part"].astype(np.float64)
    return y.astype(np.float32)
